# revision 1
# baseline (speedup 1.0000x reference)
"""BDGCN (dual-diffusion graph conv) Trainium2 kernel.

Math (per batch b):
  m1[k,m,c,l] = sum_n X[n,c,l] G[k,n,m]
  m2[m,d,k,j,l] = sum_c m1[k,m,c,l] G[j,c,d]
  out[m,d,h] = relu(sum_{k,j,l} m2[m,d,k,j,l] W[k*96+j*32+l, h] + b[h])

Sharding: data-parallel over batch; B=8 -> one batch per NeuronCore,
G/W/b replicated. No collectives.

Per-core pipeline (phase 1 bf16 operands, phases 2/3 float32r; all
psum accumulation fp32):
  Phase 1 (contract n): lhsT = X[n, c-chunk @ fixed l] bf16,
           rhs = G_k [n, m-half] bf16 -> psum [c128, m128], accum over
           2 n-chunks -> M1[k][cchk] SBUF f32r, free layout (g32,l32,r4)
           where m = 4g + r.
  Phase 2 (contract c): lhsT = M1[:, 128-col block g] (cols = (l,r)),
           rhs = G_j [c128, d256] -> psum [(l,r)128, d256], accum over
           2 c-chunks -> M2 SBUF tiles per (k,j).
  Phase 3 (contract (k,j,l)): lhsT = M2[kj][:, d-chunk],
           rhs = block-diagonal W [(l,r)128, (r,h)256] -> psum
           [d128, (m4,h)256], accum over 9 (k,j).
           Then +bias (DVE), relu (GpSimd), DMA out to [m, d, h].

Walrus-build workarounds baked in: Tile's exit drain is split into
single-wait drains (_patch_tile_drain) and any instruction carrying >1
semaphore wait gets extra waits hoisted onto NoOps (_split_multi_waits).
"""

import numpy as np

B, N, L, K, H = 8, 256, 32, 3, 64
P = 128  # partitions

_CACHE = {}


def _patch_tile_drain():
    """This container's walrus build rejects instructions carrying more
    than one semaphore wait; Tile's exit emits one drain with N waits.
    Split it into N single-wait drains."""
    import concourse.mybir as mybir
    import concourse.tile as tile

    if getattr(tile.TileContext, "_drain_split_patched", False):
        return

    def patched(self, tick_clock, wait_clock):
        from concourse.vector_clock import ScopedClock

        nc = self.nc
        probe = nc.sync.drain()
        wait_clock.add_sem_waits(
            probe.ins, ScopedClock({None: tick_clock.global_clock})
        )
        si = probe.ins.sync_info
        waits = list(si.on_wait) if si is not None else []
        if len(waits) > 1:
            si.on_wait = [waits[0]]
            for w in waits[1:]:
                d = nc.sync.drain()
                d.ins.sync_info = mybir.SyncInfo(on_update=[], on_wait=[w])
        nc.all_engine_barrier()
        assert self.sems is not None
        popped = nc._tile_sem_poison_stack.pop()
        assert popped is self._sem_poison
        nc.clear_and_free_semaphores(list(self.sems.allocated().values()))
        nc.all_engine_barrier()

    tile.TileContext._drain_and_barrier = patched
    tile.TileContext._drain_split_patched = True


def _build_nc(reps=1):
    import concourse.bass as bass
    import concourse.mybir as mybir
    import concourse.tile as tile
    from concourse import bacc

    _patch_tile_drain()

    f32 = mybir.dt.float32
    f32r = mybir.dt.float32r
    nc = bass.Bass("TRN2", target_bir_lowering=False, debug=False)

    bf16 = mybir.dt.bfloat16
    Xd = nc.dram_tensor("X", [N, N, L], bf16, kind="ExternalInput")
    Gd = nc.dram_tensor("G", [K, N, N], f32, kind="ExternalInput")
    GBd = nc.dram_tensor("GB", [K, N, N], bf16, kind="ExternalInput")
    Wr = nc.dram_tensor("WR", [K * K, P, 4 * H], f32, kind="ExternalInput")
    Bd = nc.dram_tensor("BB", [P, 4 * H], f32, kind="ExternalInput")
    Od = nc.dram_tensor("OUT", [N, N, H], f32, kind="ExternalOutput")

    NC2 = N // P  # 2 chunks of 128 along n or c
    MG = 4       # m's per group in phase 2/3
    NG = P // MG  # 32 groups per m-half

    with tile.TileContext(nc) as tc:
        with (
            tc.tile_pool(name="big", bufs=1) as big,
            tc.tile_pool(name="m2p", bufs=12) as m2p,
            tc.tile_pool(name="outp", bufs=4) as outp,
            tc.tile_pool(name="ps1", bufs=2, space="PSUM") as ps1p,
            tc.tile_pool(name="ps2", bufs=2, space="PSUM") as ps2p,
            tc.tile_pool(name="ps3", bufs=4, space="PSUM") as ps3p,
        ):
            # ---- resident loads ----
            xsb = big.tile([P, NC2 * N * L], bf16, tag="xsb")
            x4 = xsb.rearrange("p (b c l) -> p b c l", b=NC2, c=N)
            nc.sync.dma_start(
                out=x4, in_=Xd[:, :, :].rearrange("(b p) c l -> p b c l", p=P)
            )
            gsb = big.tile([P, K * NC2 * N], f32r, tag="gsb")
            g4 = gsb.rearrange("p (k b m) -> p k b m", k=K, b=NC2)
            nc.sync.dma_start(
                out=g4,
                in_=Gd[:, :, :].bitcast(f32r).rearrange(
                    "k (b p) m -> p k b m", p=P
                ),
            )
            gbsb = big.tile([P, K * NC2 * N], bf16, tag="gbsb")
            gb4 = gbsb.rearrange("p (k b m) -> p k b m", k=K, b=NC2)
            nc.sync.dma_start(
                out=gb4,
                in_=GBd[:, :, :].rearrange("k (b p) m -> p k b m", p=P),
            )
            wsb = big.tile([P, K * K * MG * H], f32r, tag="wsb")
            w3 = wsb.rearrange("p (q c) -> p q c", q=K * K)
            nc.sync.dma_start(
                out=w3,
                in_=Wr[:, :, :].bitcast(f32r).rearrange("q p c -> p q c"),
            )
            bsb = big.tile([P, MG * H], f32, tag="bsb")
            nc.sync.dma_start(out=bsb, in_=Bd[:, :])

            m1 = {}
            for k in range(K):
                for cc in range(NC2):
                    m1t = big.tile([P, L * P], f32r, tag=f"m1_{k}_{cc}", name=f"m1_{k}_{cc}")
                    m1[k, cc] = m1t

            for _rep in range(reps):
              for mh in range(2):  # m-half
                # ---- phase 1 ----
                for k in range(K):
                    for cc in range(NC2):
                        # M1 free layout: (g32, l32, r4) with m = g*4 + r
                        m1w = m1[k, cc].rearrange(
                            "p (g l r) -> p g l r", g=NG, l=L
                        )
                        for l in range(L):
                            ps = ps1p.tile([P, P], f32, tag="ps1")
                            for nchk in range(NC2):
                                nc.tensor.matmul(
                                    ps,
                                    lhsT=x4[:, nchk, cc * P:(cc + 1) * P, l],
                                    rhs=gb4[:, k, nchk, mh * P:(mh + 1) * P],
                                    start=(nchk == 0),
                                    stop=(nchk == NC2 - 1),
                                )
                            nc.vector.tensor_copy(m1w[:, :, l, :], ps)
                # ---- phases 2 + 3, per group of 4 m's ----
                for g in range(NG):
                    m2sb = {}
                    for k in range(K):
                        for j in range(K):
                            ps2 = ps2p.tile([P, N], f32, tag="ps2")
                            for cc in range(NC2):
                                lv = m1[k, cc][:, g * P:(g + 1) * P]
                                nc.tensor.matmul(
                                    ps2,
                                    lhsT=lv,
                                    rhs=g4[:, j, cc, :],
                                    start=(cc == 0),
                                    stop=(cc == NC2 - 1),
                                )
                            t = m2p.tile([P, N], f32r, tag="m2")
                            nc.vector.tensor_copy(t, ps2)
                            m2sb[k, j] = t
                    for dc in range(NC2):
                        ps3 = ps3p.tile([P, MG * H], f32, tag="ps3")
                        for idx in range(K * K):
                            k, j = divmod(idx, K)
                            nc.tensor.matmul(
                                ps3,
                                lhsT=m2sb[k, j][:, dc * P:(dc + 1) * P],
                                rhs=w3[:, idx, :],
                                start=(idx == 0),
                                stop=(idx == K * K - 1),
                            )
                        ost = outp.tile([P, MG * H], f32, tag="ost")
                        nc.vector.scalar_tensor_tensor(
                            out=ost,
                            in0=ps3,
                            scalar=0.0,
                            in1=bsb,
                            op0=mybir.AluOpType.add,
                            op1=mybir.AluOpType.add,
                        )
                        nc.gpsimd.tensor_scalar_max(ost, ost, 0.0)
                        mbase = mh * P + g * MG
                        dst = Od[mbase:mbase + MG, dc * P:(dc + 1) * P, :]
                        nc.sync.dma_start(
                            out=dst.rearrange("m d h -> d m h"), in_=ost
                        )
    _split_multi_waits(nc)
    return nc


def _split_multi_waits(nc):
    """This walrus build accepts at most one semaphore wait per
    instruction; Tile emits up to ~2-4.  Hoist extra waits onto NoOp
    instructions inserted just before, on the same engine."""
    import concourse.mybir as mybir

    n_split = 0
    for fn in nc.m.functions:
        for bb in fn.blocks:
            insts = bb.instructions
            new = []
            for inst in insts:
                si = inst.sync_info
                waits = list(si.on_wait) if si is not None else []
                if len(waits) > 1:
                    for w in waits[:-1]:
                        nop = mybir.InstNoOp(
                            name=nc.get_next_instruction_name(), ins=[], outs=[]
                        )
                        nop.engine = inst.engine
                        nop.sync_info = mybir.SyncInfo(
                            on_update=[], on_wait=[w]
                        )
                        new.append(nop)
                        n_split += 1
                    si.on_wait = [waits[-1]]
                new.append(inst)
            if n_split:
                bb.instructions = new
    return n_split


def _get_nc():
    if "nc" not in _CACHE:
        _CACHE["nc"] = _build_nc()
    return _CACHE["nc"]


def _prep(G, W, b):
    # Block-diagonal W for phase 3: rows indexed (l, r) with r = m-within-
    # group, cols (r'', h); nonzero only when r == r''.
    MG = 4
    Wbd = np.zeros((K * K, P, MG * H), dtype=np.float32)
    for k in range(K):
        for j in range(K):
            blk = W[k * (K * L) + j * L:k * (K * L) + (j + 1) * L, :]  # [L, H]
            for l in range(L):
                for r in range(MG):
                    Wbd[k * K + j, l * MG + r, r * H:(r + 1) * H] = blk[l]
    Bb = np.tile(b[None, :], (P, MG)).astype(np.float32)
    return np.ascontiguousarray(Wbd), Bb


def kernel(X, G, W, b):
    import ml_dtypes
    from concourse.bass_utils import run_bass_kernel_spmd

    X = np.ascontiguousarray(X, dtype=np.float32)
    G = np.ascontiguousarray(G, dtype=np.float32)
    W = np.ascontiguousarray(W, dtype=np.float32)
    b = np.ascontiguousarray(b, dtype=np.float32)
    nc = _get_nc()
    Wr, Bb = _prep(G, W, b)
    Xb = X.astype(ml_dtypes.bfloat16)
    Gb = G.astype(ml_dtypes.bfloat16)
    in_maps = [
        {"X": Xb[i], "G": G, "GB": Gb, "WR": Wr, "BB": Bb} for i in range(B)
    ]
    res = run_bass_kernel_spmd(nc, in_maps, list(range(B)))
    out = np.stack([res.results[i]["OUT"] for i in range(B)], axis=0)
    return out



# revision 6
# speedup vs baseline: 1.3358x; 1.3358x over previous
"""BDGCN (dual-diffusion graph conv) Trainium2 kernel.

Math (per batch b):
  m1[k,m,c,l] = sum_n X[n,c,l] G[k,n,m]
  m2[m,d,k,j,l] = sum_c m1[k,m,c,l] G[j,c,d]
  out[m,d,h] = relu(sum_{k,j,l} m2[m,d,k,j,l] W[k*96+j*32+l, h] + b[h])

Sharding: data-parallel over batch; B=8 -> one batch per NeuronCore,
G/W/b replicated. No collectives.

Per-core pipeline, all-bf16 operands with fp32 PSUM accumulation:
  Phase 1 (contract n): stationary = X[n128, c128 @ l] bf16,
           moving = G (k-paired) [n128, (k2,m256)=512] -> psum
           [c128, (k,m)], accum over 2 n-chunks; one DVE/Pool/Act copy
           per (cc,l) into m1[cc] bf16, free layout (k,g,l,r), m=4g+r.
  Phase 2 (contract c): stationary = m1[cc][k,g] (128 cols = (l,r)),
           moving = G (j-paired) [c128, (j2,d256)] -> psum
           [(l,r)128, (j,d)], accum over 2 c-chunks -> m2 SBUF bf16
           tiles [128, 768] per (g,k).
  Phase 3 (contract (k,j,l)): stationary = m2[g,k][:, j,dc-slice],
           moving = block-diagonal W [(l,r)128, (r,h)256] -> psum
           [d128, (m4,h)256], accum over 9 (k,j). Then +bias (DVE
           scalar_tensor_tensor), relu (Pool), DMA out to [m, d, h].
  Phase 3 of group g is emitted after phase 2 of group g+1 so the PE
  never waits on the psum->SBUF copies.

Walrus-build workarounds baked in: Tile's exit drain is split into
single-wait drains (_patch_tile_drain) and any instruction carrying >1
semaphore wait gets extra waits hoisted onto NoOps (_split_multi_waits).
"""

import numpy as np

B, N, L, K, H = 8, 256, 32, 3, 64
P = 128  # partitions

_CACHE = {}


def _patch_tile_drain():
    """This container's walrus build rejects instructions carrying more
    than one semaphore wait; Tile's exit emits one drain with N waits.
    Split it into N single-wait drains."""
    import concourse.mybir as mybir
    import concourse.tile as tile

    if getattr(tile.TileContext, "_drain_split_patched", False):
        return

    def patched(self, tick_clock, wait_clock):
        from concourse.vector_clock import ScopedClock

        nc = self.nc
        probe = nc.sync.drain()
        wait_clock.add_sem_waits(
            probe.ins, ScopedClock({None: tick_clock.global_clock})
        )
        si = probe.ins.sync_info
        waits = list(si.on_wait) if si is not None else []
        if len(waits) > 1:
            si.on_wait = [waits[0]]
            for w in waits[1:]:
                d = nc.sync.drain()
                d.ins.sync_info = mybir.SyncInfo(on_update=[], on_wait=[w])
        nc.all_engine_barrier()
        assert self.sems is not None
        popped = nc._tile_sem_poison_stack.pop()
        assert popped is self._sem_poison
        nc.clear_and_free_semaphores(list(self.sems.allocated().values()))
        nc.all_engine_barrier()

    tile.TileContext._drain_and_barrier = patched
    tile.TileContext._drain_split_patched = True


def _build_nc(reps=1):
    import concourse.bass as bass
    import concourse.mybir as mybir
    import concourse.tile as tile

    _patch_tile_drain()

    f32 = mybir.dt.float32
    bf16 = mybir.dt.bfloat16
    nc = bass.Bass("TRN2", target_bir_lowering=False, debug=False)

    Xd = nc.dram_tensor("X", [N, N, L], bf16, kind="ExternalInput")
    GBd = nc.dram_tensor("GB", [K, N, N], bf16, kind="ExternalInput")
    Wr = nc.dram_tensor("WR", [K * K, P, 4 * H], bf16, kind="ExternalInput")
    Bd = nc.dram_tensor("BB", [P, 4 * H], f32, kind="ExternalInput")
    Od = nc.dram_tensor("OUT", [N, N, H], f32, kind="ExternalOutput")

    NC2 = N // P  # 2 chunks of 128 along n / c / d
    MG = 4       # m's per group
    NG = N // MG  # 64 groups over all m

    add = mybir.AluOpType.add
    cp = mybir.ActivationFunctionType.Copy

    with tile.TileContext(nc) as tc:
        with (
            tc.tile_pool(name="big", bufs=1) as big,
            tc.tile_pool(name="m2p", bufs=6) as m2p,
            tc.tile_pool(name="outp", bufs=6) as outp,
            tc.tile_pool(name="psab", bufs=6, space="PSUM") as psabp,
            tc.tile_pool(name="ps3", bufs=2, space="PSUM") as ps3p,
        ):
            # ---- resident loads ----
            xsb = big.tile([P, NC2 * N * L], bf16, tag="xsb")
            x4 = xsb.rearrange("p (b c l) -> p b c l", b=NC2, c=N)
            nc.sync.dma_start(
                out=x4, in_=Xd[:, :, :].rearrange("(b p) c l -> p b c l", p=P)
            )
            gsb = big.tile([P, NC2 * K * N], bf16, tag="gsb")
            g4 = gsb.rearrange("p (b k m) -> p b k m", b=NC2, k=K)
            for k in range(K):
                nc.sync.dma_start(
                    out=g4[:, :, k, :],
                    in_=GBd[k, :, :].rearrange("(b p) m -> p b m", p=P),
                )
            wsb = big.tile([P, K * K * MG * H], bf16, tag="wsb")
            w3 = wsb.rearrange("p (q c) -> p q c", q=K * K)
            nc.sync.dma_start(
                out=w3, in_=Wr[:, :, :].rearrange("q p c -> p q c")
            )
            bsb = big.tile([P, MG * H], f32, tag="bsb")
            nc.sync.dma_start(out=bsb, in_=Bd[:, :])

            m1 = {}
            for cc in range(NC2):
                t = big.tile(
                    [P, K * NG * L * MG], bf16,
                    tag=f"m1_{cc}", name=f"m1_{cc}",
                )
                m1[cc] = t.rearrange(
                    "p (k g l r) -> p k g l r", k=K, g=NG, l=L
                )

            cp_state = [0]

            def copy_on(out, in_):
                # psum -> SBUF; only DVE and Act can read PSUM
                e = cp_state[0] % 2
                cp_state[0] += 1
                if e == 0:
                    nc.vector.tensor_copy(out, in_)
                else:
                    nc.scalar.activation(out, in_, cp)

            def phase1():
                for cc in range(NC2):
                    for l in range(L):
                        ps = psabp.tile([P, 512], f32, tag="psab")
                        psb = psabp.tile([P, 512], f32, tag="psab")
                        for nchk in range(NC2):
                            st = x4[:, nchk, cc * P:(cc + 1) * P, l]
                            nc.tensor.matmul(
                                ps, lhsT=st, rhs=g4[:, nchk, 0:2, :],
                                start=(nchk == 0), stop=(nchk == 1),
                            )
                            nc.tensor.matmul(
                                psb[:, 0:N], lhsT=st, rhs=g4[:, nchk, 2, :],
                                start=(nchk == 0), stop=(nchk == 1),
                            )
                        # psum cols (k, m), m = 4g + r -> m1[cc][k, g, l, r]
                        copy_on(
                            m1[cc][:, 0:2, :, l, :],
                            ps.rearrange("p (k g r) -> p k g r", k=2, g=NG),
                        )
                        copy_on(
                            m1[cc][:, 2, :, l, :],
                            psb[:, 0:N].rearrange("p (g r) -> p g r", g=NG),
                        )

            def p2(g):
                tiles = []
                for k in range(K):
                    ps = psabp.tile([P, 512], f32, tag="psab")
                    psb = psabp.tile([P, 512], f32, tag="psab")
                    for cc in range(NC2):
                        st = m1[cc][:, k, g, :, :]  # 128 cols = (l, r)
                        nc.tensor.matmul(
                            ps, lhsT=st, rhs=g4[:, cc, 0:2, :],
                            start=(cc == 0), stop=(cc == 1),
                        )
                        nc.tensor.matmul(
                            psb[:, 0:N], lhsT=st, rhs=g4[:, cc, 2, :],
                            start=(cc == 0), stop=(cc == 1),
                        )
                    t = m2p.tile([P, K * N], bf16, tag="m2")
                    copy_on(t[:, 0:512], ps)
                    copy_on(t[:, 512:768], psb[:, 0:N])
                    tiles.append(t)
                return tiles

            def p3(g, tiles):
                for dc in range(NC2):
                    ps3 = ps3p.tile([P, 512], f32, tag="ps3")
                    for idx in range(K * K):
                        k, j = divmod(idx, K)
                        nc.tensor.matmul(
                            ps3[:, 0:MG * H],
                            lhsT=tiles[k][:, j * N + dc * P:
                                          j * N + (dc + 1) * P],
                            rhs=w3[:, idx, :],
                            start=(idx == 0), stop=(idx == K * K - 1),
                        )
                    ost = outp.tile([P, MG * H], f32, tag="ost")
                    copy_on(ost, ps3[:, 0:MG * H])
                    nc.gpsimd.tensor_add(ost, ost, bsb)
                    nc.gpsimd.tensor_scalar_max(ost, ost, 0.0)
                    mbase = g * MG
                    dst = Od[mbase:mbase + MG, dc * P:(dc + 1) * P, :]
                    nc.sync.dma_start(
                        out=dst.rearrange("m d h -> d m h"), in_=ost
                    )

            for _rep in range(reps):
                phase1()
                prev = None
                for g in range(NG):
                    cur = p2(g)
                    if prev is not None:
                        p3(g - 1, prev)
                    prev = cur
                p3(NG - 1, prev)

    _split_multi_waits(nc)
    return nc


def _split_multi_waits(nc):
    """This walrus build accepts at most one semaphore wait per
    instruction; Tile emits up to ~2-4.  Hoist extra waits onto NoOp
    instructions inserted just before, on the same engine."""
    import concourse.mybir as mybir

    n_split = 0
    for fn in nc.m.functions:
        for bb in fn.blocks:
            insts = bb.instructions
            new = []
            for inst in insts:
                si = inst.sync_info
                waits = list(si.on_wait) if si is not None else []
                if len(waits) > 1:
                    for w in waits[:-1]:
                        nop = mybir.InstNoOp(
                            name=nc.get_next_instruction_name(), ins=[], outs=[]
                        )
                        nop.engine = inst.engine
                        nop.sync_info = mybir.SyncInfo(
                            on_update=[], on_wait=[w]
                        )
                        new.append(nop)
                        n_split += 1
                    si.on_wait = [waits[-1]]
                new.append(inst)
            if n_split:
                bb.instructions = new
    return n_split


def _get_nc():
    if "nc" not in _CACHE:
        _CACHE["nc"] = _build_nc()
    return _CACHE["nc"]


def _prep(G, W, b):
    # Block-diagonal W for phase 3: rows indexed (l, r) with r = m-within-
    # group, cols (r'', h); nonzero only when r == r''.
    MG = 4
    Wbd = np.zeros((K * K, P, MG * H), dtype=np.float32)
    for k in range(K):
        for j in range(K):
            blk = W[k * (K * L) + j * L:k * (K * L) + (j + 1) * L, :]  # [L, H]
            for l in range(L):
                for r in range(MG):
                    Wbd[k * K + j, l * MG + r, r * H:(r + 1) * H] = blk[l]
    Bb = np.tile(b[None, :], (P, MG)).astype(np.float32)
    return np.ascontiguousarray(Wbd), Bb


def _make_in_maps(X, G, W, b):
    import ml_dtypes

    X = np.ascontiguousarray(X, dtype=np.float32)
    G = np.ascontiguousarray(G, dtype=np.float32)
    W = np.ascontiguousarray(W, dtype=np.float32)
    b = np.ascontiguousarray(b, dtype=np.float32)
    Wr, Bb = _prep(G, W, b)
    Xb = X.astype(ml_dtypes.bfloat16)
    Gb = G.astype(ml_dtypes.bfloat16)
    Wrb = Wr.astype(ml_dtypes.bfloat16)
    return [
        {"X": Xb[i], "GB": Gb, "WR": Wrb, "BB": Bb} for i in range(B)
    ]


def kernel(X, G, W, b):
    from concourse.bass_utils import run_bass_kernel_spmd

    nc = _get_nc()
    in_maps = _make_in_maps(X, G, W, b)
    res = run_bass_kernel_spmd(nc, in_maps, list(range(B)))
    out = np.stack([res.results[i]["OUT"] for i in range(B)], axis=0)
    return out


# revision 23
# speedup vs baseline: 2.3629x; 1.7689x over previous
"""BDGCN (dual-diffusion graph conv) Trainium2 kernel.

Math (per batch b):
  m1[k,m,c,l] = sum_n X[n,c,l] G[k,n,m]
  m2[m,d,k,j,l] = sum_c m1[k,m,c,l] G[j,c,d]
  out[m,d,h] = relu(sum_{k,j,l} m2[m,d,k,j,l] W[k*96+j*32+l, h] + b[h])

Sharding: data-parallel over batch; B=8 -> one batch per NeuronCore,
G/W/b replicated. No collectives.

Per-core pipeline, all-bf16 operands with fp32 PSUM accumulation:
  Phase 1 (contract n): stationary = X[n128, c128 @ l] bf16,
           moving = G (k-paired) [n128, (k2,m256)=512] -> psum
           [c128, (k,m)], accum over 2 n-chunks; one DVE/Pool/Act copy
           per (cc,l) into m1[cc] bf16, free layout (k,g,l,r), m=4g+r.
  Phase 2 (contract c): stationary = m1[cc][k,g] (128 cols = (l,r)),
           moving = G (j-paired) [c128, (j2,d256)] -> psum
           [(l,r)128, (j,d)], accum over 2 c-chunks -> m2 SBUF bf16
           tiles [128, 768] per (g,k).
  Phase 3 (contract (k,j,l)): stationary = m2[g,k][:, j,dc-slice],
           moving = block-diagonal W [(l,r)128, (r,h)256] -> psum
           [d128, (m4,h)256], accum over 9 (k,j). Then +bias (DVE
           scalar_tensor_tensor), relu (Pool), DMA out to [m, d, h].
  Phase 3 of group g is emitted after phase 2 of group g+1 so the PE
  never waits on the psum->SBUF copies.

Walrus-build workarounds baked in: Tile's exit drain is split into
single-wait drains (_patch_tile_drain) and any instruction carrying >1
semaphore wait gets extra waits hoisted onto NoOps (_split_multi_waits).
"""

import numpy as np

B, N, L, K, H = 8, 256, 32, 3, 64
P = 128  # partitions

_CACHE = {}


def _patch_tile_drain():
    """This container's walrus build rejects instructions carrying more
    than one semaphore wait; Tile's exit emits one drain with N waits.
    Split it into N single-wait drains."""
    import concourse.mybir as mybir
    import concourse.tile as tile

    if getattr(tile.TileContext, "_drain_split_patched", False):
        return

    def patched(self, tick_clock, wait_clock):
        from concourse.vector_clock import ScopedClock

        nc = self.nc
        probe = nc.sync.drain()
        wait_clock.add_sem_waits(
            probe.ins, ScopedClock({None: tick_clock.global_clock})
        )
        si = probe.ins.sync_info
        waits = list(si.on_wait) if si is not None else []
        if len(waits) > 1:
            si.on_wait = [waits[0]]
            for w in waits[1:]:
                d = nc.sync.drain()
                d.ins.sync_info = mybir.SyncInfo(on_update=[], on_wait=[w])
        nc.all_engine_barrier()
        assert self.sems is not None
        popped = nc._tile_sem_poison_stack.pop()
        assert popped is self._sem_poison
        nc.clear_and_free_semaphores(list(self.sems.allocated().values()))
        nc.all_engine_barrier()

    tile.TileContext._drain_and_barrier = patched
    tile.TileContext._drain_split_patched = True


def _build_nc(reps=1, mode="full", split_waits=True):
    import concourse.bass as bass
    import concourse.mybir as mybir
    import concourse.tile as tile

    _patch_tile_drain()

    f32 = mybir.dt.float32
    bf16 = mybir.dt.bfloat16
    nc = bass.Bass("TRN2", target_bir_lowering=False, debug=False)

    Xd = nc.dram_tensor("X", [N, N, L], bf16, kind="ExternalInput")
    GBd = nc.dram_tensor("GB", [K, N, N], bf16, kind="ExternalInput")
    Wr = nc.dram_tensor("WR", [K * K, P, 4 * H], bf16, kind="ExternalInput")
    Bd = nc.dram_tensor("BR", [1, 8 * H], bf16, kind="ExternalInput")
    Od = nc.dram_tensor("OUT", [N, N, H], f32, kind="ExternalOutput")

    NC2 = N // P  # 2 chunks of 128 along n / c / d
    MG = 4       # m's per group
    NG = N // MG  # 64 groups over all m

    cp = mybir.ActivationFunctionType.Copy
    relu = mybir.ActivationFunctionType.Relu

    with tile.TileContext(nc) as tc:
        with (
            tc.tile_pool(name="big", bufs=1) as big,
            tc.tile_pool(name="m2p", bufs=12) as m2p,
            tc.tile_pool(name="outp", bufs=4) as outp,
            tc.tile_pool(name="tp", bufs=6, space="PSUM") as tpp,
            tc.tile_pool(name="ps3", bufs=2, space="PSUM") as ps3p,
        ):
            # ---- resident loads ----
            xsb = big.tile([P, NC2 * N * L], bf16, tag="xsb")
            x4 = xsb.rearrange("p (b c l) -> p b c l", b=NC2, c=N)
            nc.sync.dma_start(
                out=x4, in_=Xd[:, :, :].rearrange("(b p) c l -> p b c l", p=P)
            )
            gsb = big.tile([P, NC2 * K * N], bf16, tag="gsb")
            g4 = gsb.rearrange("p (b k m) -> p b k m", b=NC2, k=K)
            for k in range(K):
                nc.sync.dma_start(
                    out=g4[:, :, k, :],
                    in_=GBd[k, :, :].rearrange("(b p) m -> p b m", p=P),
                )
            wsb = big.tile([P, K * K * MG * H], bf16, tag="wsb")
            w3 = wsb.rearrange("p (q c) -> p q c", q=K * K)
            nc.sync.dma_start(
                out=w3, in_=Wr[:, :, :].rearrange("q p c -> p q c")
            )
            brow = big.tile([1, 8 * H], bf16, tag="brow")
            nc.sync.dma_start(out=brow, in_=Bd[:, :])
            ones = big.tile([1, P], bf16, tag="ones")
            nc.gpsimd.memset(ones[:, :], 1.0)

            m1 = {}
            for cc in range(NC2):
                t = big.tile(
                    [P, K * NG * L * MG], bf16,
                    tag=f"m1_{cc}", name=f"m1_{cc}",
                )
                if mode == "pe_only":
                    nc.gpsimd.memset(t[:, :], 0.25)
                m1[cc] = t.rearrange(
                    "p (k g l r) -> p k g l r", k=K, g=NG, l=L
                )

            static_m2 = None
            if mode == "pe_only":
                static_m2 = big.tile([P, 512], bf16, tag="sm2", name="sm2")
                nc.gpsimd.memset(static_m2[:, :], 0.25)

            cp_state = [0]

            def copy_on(out, in_):
                # psum -> SBUF; only DVE and Act can read PSUM
                if mode == "pe_only":
                    return
                e = cp_state[0] % 2
                cp_state[0] += 1
                if e == 0:
                    nc.vector.tensor_copy(out, in_)
                else:
                    nc.scalar.activation(out, in_, cp)

            def phase1():
                for cc in range(NC2):
                    bpend = None
                    for l in range(L):
                        pa = tpp.tile([P, 512], f32, tag="tp", name="pa")
                        if bpend is None:
                            bpend = tpp.tile([P, 512], f32, tag="tp", name="pb")
                            boff = 0
                        else:
                            boff = N
                        for nchk in range(NC2):
                            st = x4[:, nchk, cc * P:(cc + 1) * P, l]
                            nc.tensor.matmul(
                                pa, lhsT=st, rhs=g4[:, nchk, 0:2, :],
                                start=(nchk == 0), stop=(nchk == 1),
                                skip_group_check=True,
                            )
                            nc.tensor.matmul(
                                bpend[:, boff:boff + N], lhsT=st,
                                rhs=g4[:, nchk, 2, :],
                                start=(nchk == 0), stop=(nchk == 1),
                                skip_group_check=True,
                            )
                        # psum cols (k,m), m = 4g+r -> m1[cc][k, g, l, r]
                        copy_on(
                            m1[cc][:, 0:2, :, l, :],
                            pa.rearrange("p (k g r) -> p k g r", k=2, g=NG),
                        )
                        if boff == N:
                            copy_on(
                                m1[cc][:, 2, :, l - 1:l + 1, :],
                                bpend.rearrange(
                                    "p (h g r) -> p g h r", h=2, g=NG
                                ),
                            )
                            bpend = None

            # ---- phases 2 + 3, interleaved at (k,j) granularity ----
            st2 = {}

            def p2_reset():
                st2.update(pend=None, m2=[], nq=0)

            def p2_unit(g, q):
                k, j = divmod(q, K)
                if st2["pend"] is None:
                    st2["pend"] = tpp.tile([P, 512], f32, tag="tp", name="t2")
                    off = 0
                else:
                    off = N
                T = st2["pend"]
                for cc in range(NC2):
                    nc.tensor.matmul(
                        T[:, off:off + N],
                        lhsT=m1[cc][:, k, g, :, :],  # 128 cols = (l, r)
                        rhs=g4[:, cc, j, :],
                        start=(cc == 0), stop=(cc == 1),
                        skip_group_check=True,
                    )
                if off == N:
                    if mode == "pe_only":
                        st2["m2"].append(static_m2)
                        st2["m2"].append(static_m2)
                    else:
                        t = m2p.tile([P, 512], bf16, tag="m2", name="m2t")
                        copy_on(t, T)
                        st2["m2"].append(t)
                        st2["m2"].append(t)
                    st2["pend"] = None

            def p3_start(ps3):
                # rank-1 bias seed over both dc halves: ps3 = ones^T @ brow.
                # start=True zeroes the whole psum bank once; every later
                # (k,j) matmul accumulates with start=False.
                nc.tensor.matmul(
                    ps3, lhsT=ones, rhs=brow,
                    start=True, stop=False, skip_group_check=True,
                )

            def p3_unit(g, q, ps3):
                qg = g * K * K + q
                t = st2["m2"][qg]
                base = (qg % 2) * N
                for dc in range(NC2):
                    nc.tensor.matmul(
                        ps3[:, dc * N:dc * N + MG * H],
                        lhsT=t[:, base + dc * P:base + (dc + 1) * P],
                        rhs=w3[:, q, :],
                        start=False, stop=(q == K * K - 1 and dc == NC2 - 1),
                        skip_group_check=True,
                    )

            def p3_finish(g, ps3):
                if mode == "pe_only":
                    return
                ost = outp.tile([P, 512], f32, tag="ost", name="ost")
                if cp_state[0] % 2 == 0:
                    nc.vector.tensor_scalar_max(ost, ps3, 0.0)
                else:
                    nc.scalar.activation(ost, ps3, relu)
                cp_state[0] += 1
                if mode == "no_dma":
                    return
                mbase = g * MG
                for dc in range(NC2):
                    dst = Od[mbase:mbase + MG, dc * P:(dc + 1) * P, :]
                    nc.sync.dma_start(
                        out=dst.rearrange("m d h -> d m h"),
                        in_=ost[:, dc * N:(dc + 1) * N],
                    )

            for _rep in range(reps):
                phase1()
                p2_reset()
                ps3 = None
                for g in range(NG):
                    if g > 0:
                        ps3 = ps3p.tile([P, 512], f32, tag="ps3", name="ps3t")
                        p3_start(ps3)
                    for q in range(K * K):
                        p2_unit(g, q)
                        if g > 0:
                            p3_unit(g - 1, q, ps3)
                    if g > 0:
                        p3_finish(g - 1, ps3)
                ps3 = ps3p.tile([P, 512], f32, tag="ps3", name="ps3t")
                p3_start(ps3)
                for q in range(K * K):
                    p3_unit(NG - 1, q, ps3)
                p3_finish(NG - 1, ps3)

    if split_waits:
        _split_multi_waits(nc)
    return nc


def _split_multi_waits(nc):
    """This walrus build accepts at most one semaphore wait per
    instruction; Tile emits up to ~2-4.  Hoist extra waits onto NoOp
    instructions inserted just before, on the same engine."""
    import concourse.mybir as mybir

    n_split = 0
    for fn in nc.m.functions:
        for bb in fn.blocks:
            insts = bb.instructions
            new = []
            for inst in insts:
                si = inst.sync_info
                waits = list(si.on_wait) if si is not None else []
                if len(waits) > 1:
                    for w in waits[:-1]:
                        nop = mybir.InstNoOp(
                            name=nc.get_next_instruction_name(), ins=[], outs=[]
                        )
                        nop.engine = inst.engine
                        nop.sync_info = mybir.SyncInfo(
                            on_update=[], on_wait=[w]
                        )
                        new.append(nop)
                        n_split += 1
                    si.on_wait = [waits[-1]]
                new.append(inst)
            if n_split:
                bb.instructions = new
    return n_split


def _get_nc():
    if "nc" not in _CACHE:
        _CACHE["nc"] = _build_nc()
    return _CACHE["nc"]


def _prep(G, W, b):
    # Block-diagonal W for phase 3: rows indexed (l, r) with r = m-within-
    # group, cols (r'', h); nonzero only when r == r''.
    MG = 4
    Wbd = np.zeros((K * K, P, MG * H), dtype=np.float32)
    for k in range(K):
        for j in range(K):
            blk = W[k * (K * L) + j * L:k * (K * L) + (j + 1) * L, :]  # [L, H]
            for l in range(L):
                for r in range(MG):
                    Wbd[k * K + j, l * MG + r, r * H:(r + 1) * H] = blk[l]
    Br = np.tile(b, 8)[None, :].astype(np.float32)
    return np.ascontiguousarray(Wbd), Br


def _make_in_maps(X, G, W, b):
    import ml_dtypes

    X = np.ascontiguousarray(X, dtype=np.float32)
    G = np.ascontiguousarray(G, dtype=np.float32)
    W = np.ascontiguousarray(W, dtype=np.float32)
    b = np.ascontiguousarray(b, dtype=np.float32)
    Wr, Br = _prep(G, W, b)
    Xb = X.astype(ml_dtypes.bfloat16)
    Gb = G.astype(ml_dtypes.bfloat16)
    Wrb = Wr.astype(ml_dtypes.bfloat16)
    Brb = Br.astype(ml_dtypes.bfloat16)
    return [
        {"X": Xb[i], "GB": Gb, "WR": Wrb, "BR": Brb} for i in range(B)
    ]


def kernel(X, G, W, b):
    from concourse.bass_utils import run_bass_kernel_spmd

    nc = _get_nc()
    in_maps = _make_in_maps(X, G, W, b)
    res = run_bass_kernel_spmd(nc, in_maps, list(range(B)))
    out = np.stack([res.results[i]["OUT"] for i in range(B)], axis=0)
    return out
